# revision 3
# baseline (speedup 1.0000x reference)
"""Joseph 3D projector on 8 TRN2 NeuronCores — banded-matmul, mirror-shared M.

Formulation: for each angle a, out[u, v] = DT * sum_p M_a[p, u] * volT[p, v]
where p = y*128 + x and volT[p, v] = vol[0,0,x,y,v] (the reference's
z-interpolation is an exact identity for this geometry). M_a is ~1.5% dense:
for a fixed contraction slab (a y-line or x-line of the volume) the nonzero
u's lie in a narrow window, so we stream only per-slab windows in fp8-e3m4
and issue one small matmul per (slab, angle-slot) accumulating at the
window's column offset in PSUM.

v2 over the baseline:
  * Mirror sharing: the angle involution s(k) = (60-k) mod 120 maps M_k to
    M_{s(k)} by an EXACT x<->y transpose (k in [61,119], plain) or
    transpose + u-flip (k in [0,60]).  Slots are paired so the mirrored slot
    reads the SAME SBUF m-block (u-flip pairs via a negative-stride rhs AP).
    Stored M drops 13.5 MB -> 9.1 MB per core.
  * LDWEIGHTS elision: one explicit ldweights per (phase, slab); all the
    slab's matmuls carry InstMatmult.ldweights=False (verified exact on HW).
  * No SWDGE: psum drain split DVE/ACT, out flushed on the two HWDGE queues
    after all stream dma_starts are in the rings.

Sharding: 15 angle-slots; slot j on core c holds angle ANG[j][c]; all 8
cores share one SPMD program whose per-slot window geometry (width, per-slab
column offsets) covers the slot's 8 angles.
"""
import numpy as np
import ml_dtypes

D = H = W = 128
V = U = 128
A = 120
S = 128
NCORES = 8
NSLOT = A // NCORES  # 15
T = 0.5 * float(np.sqrt(((W - 1) * 1.0) ** 2 + ((H - 1) * 1.0) ** 2))
DT = 2.0 * T / S

ELIDE_LDW = True

# ---- slot table ------------------------------------------------------------
# Each entry: (angles per core c, stored-block id or (block id, 'plain'|'flip'))
# s(k) = (60-k) mod 120 pairs: k in [0,60]: transpose+uflip; [61,119]: transpose.
def _slot_table():
    slots = []
    # zone-B pairs (base axis 0, mirror axis 1, u-flip sharing)
    for b, a0 in enumerate((0, 8, 16)):
        slots.append((tuple(a0 + c for c in range(8)), b, None))
    # zone-A pairs (base axis 1, mirror axis 0, plain sharing)
    for b, a0 in zip((3, 4, 5), (61, 69, 77)):
        slots.append((tuple(a0 + c for c in range(8)), b, None))
    # unpaired
    slots.append((tuple(24 + c for c in range(8)), 6, None))     # U0 axis0
    slots.append((tuple(85 + c for c in range(8)), 7, None))     # U1 axis1
    slots.append(((32, 33, 34, 35, 36, 93, 94, 95), 8, None))    # U2 axis1
    # mirrors
    for j, kind in ((0, "flip"), (1, "flip"), (2, "flip"),
                    (3, "plain"), (4, "plain"), (5, "plain")):
        base_angs = slots[j][0]
        m_angs = tuple((60 - k) % 120 for k in base_angs)
        slots.append((m_angs, slots[j][1], kind))
    return slots

SLOTS = _slot_table()  # 9 base slots then 6 mirror slots


def _build_M(cos_t, sin_t):
    """Dense M[p=(y*W+x), u] float32 for one angle."""
    u_phys = np.arange(U, dtype=np.float64) - (U - 1) / 2.0
    t = -T + (np.arange(S, dtype=np.float64) + 0.5) * DT
    x_idx = (-u_phys[None, :] * sin_t + t[:, None] * cos_t) + (W - 1) / 2.0
    y_idx = (u_phys[None, :] * cos_t + t[:, None] * sin_t) + (H - 1) / 2.0
    x0 = np.floor(x_idx).astype(np.int64)
    y0 = np.floor(y_idx).astype(np.int64)
    wx = x_idx - x0
    wy = y_idx - y0
    Mflat = np.zeros(H * W * U, np.float32)
    uu = np.broadcast_to(np.arange(U, dtype=np.int64)[None, :], (S, U))
    for dy, dx in ((0, 0), (0, 1), (1, 0), (1, 1)):
        yi = y0 + dy
        xi = x0 + dx
        w = (wy if dy else 1 - wy) * (wx if dx else 1 - wx)
        valid = (xi >= 0) & (xi <= W - 1) & (yi >= 0) & (yi <= H - 1)
        p = np.clip(yi, 0, H - 1) * W + np.clip(xi, 0, W - 1)
        flat = (p * U + uu)[valid]
        Mflat += np.bincount(flat, weights=w[valid].astype(np.float64),
                             minlength=H * W * U).astype(np.float32)
    return Mflat.reshape(H * W, U)


def _slot_axis(angs):
    th = np.array(angs, np.float64) * (np.pi / A)
    return 0 if np.mean(np.abs(np.sin(th))) <= np.mean(np.abs(np.cos(th))) else 1


def _union_geom(Ms, angs, ax):
    """Union band windows over a slot's 8 angles: (widths[S], lo[S])."""
    lo = np.full(S, U, np.int64)
    hi = np.full(S, -1, np.int64)
    for k in angs:
        Mr = Ms[k].reshape(H, W, U)
        sl = Mr if ax == 0 else Mr.transpose(1, 0, 2)  # [slab, kdim, u]
        nz = sl.any(axis=1)
        any_s = nz.any(axis=1)
        first = nz.argmax(axis=1)
        last = U - 1 - nz[:, ::-1].argmax(axis=1)
        lo = np.where(any_s, np.minimum(lo, first), lo)
        hi = np.where(any_s, np.maximum(hi, last), hi)
    wv = np.maximum(hi - lo + 1, 0).astype(np.int64)
    lo = np.where(hi < 0, 0, lo).astype(np.int64)
    return wv, lo


def _schedule(angles):
    """Per-slot geometry. Returns Ms plus per-slot (axis, widths, offs) and
    per-slot m-block reference (block id, flip?)."""
    Ms = np.stack([_build_M(np.cos(np.float64(a)), np.sin(np.float64(a)))
                   for a in angles])
    axes, widths, offs = [], [], []
    for j, (angs, blk, kind) in enumerate(SLOTS):
        ax = _slot_axis(angs)
        wv, lo = _union_geom(Ms, angs, ax)
        if kind is not None:
            base_wv, base_lo = widths[blk], offs[blk]
            assert (wv == base_wv).all(), f"slot {j}: mirror width mismatch"
            if kind == "flip":
                exp_lo = np.where(base_wv > 0, U - base_lo - base_wv, lo)
            else:
                exp_lo = base_lo
            assert (np.where(wv > 0, lo, exp_lo) == np.where(
                wv > 0, exp_lo, lo)).all(), f"slot {j}: mirror offs mismatch"
            lo = exp_lo
        cover = np.zeros(U, bool)
        for s in range(S):
            if wv[s] > 0:
                cover[lo[s]:lo[s] + wv[s]] = True
        assert cover.all(), f"slot {j}: uncovered psum cols"
        axes.append(ax)
        widths.append(wv)
        offs.append(lo)
    return Ms, np.array(axes), np.stack(widths), np.stack(offs)


# stored blocks by need-phase: stream0 = blocks whose first use is phase 0
# (bases 0,1,2 + bases 3,4,5 via their phase-0 mirrors + U0); stream1 = U1,U2.
STREAM0_BLOCKS = [0, 1, 2, 3, 4, 5, 6]
STREAM1_BLOCKS = [7, 8]
BLOCK_BASE_SLOT = {b: j for j, (angs, b_, kind) in enumerate(SLOTS)
                   if kind is None for b in [b_]}


def _layout(widths, blocks):
    """Slab-major packed columns for stored blocks: base[s], cum[blk][s]."""
    base = np.zeros(S + 1, np.int64)
    cum = {b: np.zeros(S, np.int64) for b in blocks}
    for s in range(S):
        c = 0
        for b in blocks:
            cum[b][s] = c
            c += int(widths[BLOCK_BASE_SLOT[b]][s])
        base[s + 1] = base[s] + c
    return base, cum, int(base[S])


def _pack_core(Ms, axes, widths, offs, lay0, lay1, core):
    """fp8 stored blocks for one core."""
    f8 = ml_dtypes.float8_e3m4
    base0, cum0, W0tot = lay0
    base1, cum1, W1tot = lay1
    m0 = np.zeros((S, W0tot), f8)
    m1 = np.zeros((S, W1tot), f8)
    for b in range(9):
        j = BLOCK_BASE_SLOT[b]
        angs = SLOTS[j][0]
        k = angs[core]
        ax = axes[j]
        Mr = Ms[k].reshape(H, W, U)
        sl = Mr if ax == 0 else Mr.transpose(1, 0, 2)
        if b in STREAM0_BLOCKS:
            m, base, cum = m0, base0, cum0[b]
        else:
            m, base, cum = m1, base1, cum1[b]
        for s in range(S):
            w = int(widths[j][s])
            if w == 0:
                continue
            o = int(offs[j][s])
            c0 = int(base[s] + cum[s])
            m[:, c0:c0 + w] = sl[s][:, o:o + w].astype(f8)
    return m0, m1


_COMPILED = {}


def _get_compiled(angles):
    key = hash(angles.tobytes())
    if key in _COMPILED:
        return _COMPILED[key]
    from contextlib import ExitStack
    import concourse.bacc as bacc
    import concourse.tile as tile
    import concourse.mybir as mybir

    Ms, axes, widths, offs = _schedule(angles)
    phase0_slots = [j for j in range(NSLOT) if axes[j] == 0]
    phase1_slots = [j for j in range(NSLOT) if axes[j] == 1]
    assert len(phase0_slots) == 7 and len(phase1_slots) == 8
    order = phase0_slots + phase1_slots          # psum/out column order
    banks = [order[0:4], order[4:7], order[7:11], order[11:15]]
    bank_of = {j: b for b, bs in enumerate(banks) for j in bs}
    block_of = {j: bs.index(j) for bs in banks for j in bs}
    lay0 = _layout(widths, STREAM0_BLOCKS)
    lay1 = _layout(widths, STREAM1_BLOCKS)
    base0, cum0, W0tot = lay0
    base1, cum1, W1tot = lay1

    # out column base per slot (bank-major, 128 cols per slot)
    out_base = {}
    col = 0
    for bs in banks:
        for j in bs:
            out_base[j] = col
            col += U
    assert col == NSLOT * U

    nc = bacc.Bacc("TRN2", target_bir_lowering=False, debug=False,
                   enable_asserts=False, num_devices=NCORES)
    bf16 = mybir.dt.bfloat16
    f8 = mybir.dt.float8e3
    f32 = mybir.dt.float32

    volA_d = nc.dram_tensor("volA", [S, H * D], bf16, kind="ExternalInput").ap()
    volB_d = nc.dram_tensor("volB", [S, H * D], bf16, kind="ExternalInput").ap()
    m0_d = nc.dram_tensor("m0", [S, W0tot], f8, kind="ExternalInput").ap()
    m1_d = nc.dram_tensor("m1", [S, W1tot], f8, kind="ExternalInput").ap()
    out_d = nc.dram_tensor("out", [V, NSLOT * U], f32, kind="ExternalOutput").ap()

    with tile.TileContext(nc) as tc:
        with ExitStack() as ctx:
            sbuf = ctx.enter_context(tc.tile_pool(name="sbuf", bufs=1))
            psum = ctx.enter_context(tc.tile_pool(name="psum", bufs=1, space="PSUM"))

            volA_sb = sbuf.tile([S, H * D], bf16)
            volB_sb = sbuf.tile([S, H * D], bf16)
            m0_sb = sbuf.tile([S, W0tot], f8)
            m1_sb = sbuf.tile([S, W1tot], f8)
            out_sb = sbuf.tile([V, NSLOT * U], f32)
            ps = [psum.tile([V, 512], f32, name=f"ps{b}") for b in range(4)]

            # ---- stream DMAs: all issued up-front on the two HWDGE rings,
            # graded chunks in need order, greedily byte-balanced.
            mb = [0, 2, 4, 8, 16, 32, 48, 64, 96, 128]
            mchunks = list(zip(mb[:-1], mb[1:]))
            vb = [0, 4, 8, 16, 32, 64, 96, 128]
            vchunks = list(zip(vb[:-1], vb[1:]))

            items = []  # (need, bytes, dst, src)
            for s0, s1 in vchunks:
                items.append((s0, (s1 - s0) * H * S * 2,
                              volA_sb[:, s0 * H:s1 * H], volA_d[:, s0 * H:s1 * H]))
            for s0, s1 in mchunks:
                c0, c1 = int(base0[s0]), int(base0[s1])
                if c1 > c0:
                    items.append((s0, (c1 - c0) * S,
                                  m0_sb[:, c0:c1], m0_d[:, c0:c1]))
            for s0, s1 in vchunks:
                items.append((S + s0, (s1 - s0) * H * S * 2,
                              volB_sb[:, s0 * H:s1 * H], volB_d[:, s0 * H:s1 * H]))
            for s0, s1 in mchunks:
                c0, c1 = int(base1[s0]), int(base1[s1])
                if c1 > c0:
                    items.append((S + s0, (c1 - c0) * S,
                                  m1_sb[:, c0:c1], m1_d[:, c0:c1]))
            items.sort(key=lambda it: it[0])
            qbytes = [0, 0]
            qeng = [nc.sync, nc.scalar]
            for _, nb, dst, src in items:
                qi = 0 if qbytes[0] <= qbytes[1] else 1
                qeng[qi].dma_start(dst, src)
                qbytes[qi] += nb

            # ---- rhs AP for a slot at slab s
            def rhs_ap(j, s):
                blk = SLOTS[j][1]
                kind = SLOTS[j][2]
                jb = BLOCK_BASE_SLOT[blk]
                w = int(widths[jb][s])
                if blk in STREAM0_BLOCKS:
                    m_sb, base, cum = m0_sb, base0, cum0[blk]
                else:
                    m_sb, base, cum = m1_sb, base1, cum1[blk]
                c0 = int(base[s] + cum[s])
                ap = m_sb[:, c0:c0 + w]
                if kind == "flip":
                    ap = ap[:, ::-1]
                return ap

            # program-order first/last matmul per psum bank for start/stop
            emitted = []
            for phase, slots in ((0, phase0_slots), (1, phase1_slots)):
                for s in range(S):
                    for j in slots:
                        if int(widths[BLOCK_BASE_SLOT[SLOTS[j][1]]][s]) > 0:
                            emitted.append(bank_of[j])
            first_of_bank = {}
            last_of_bank = {}
            for i, b in enumerate(emitted):
                if b not in first_of_bank:
                    first_of_bank[b] = i
                last_of_bank[b] = i

            i = 0
            for phase, slots in ((0, phase0_slots), (1, phase1_slots)):
                vol_sb = volA_sb if phase == 0 else volB_sb
                for s in range(S):
                    ws = [int(widths[BLOCK_BASE_SLOT[SLOTS[j][1]]][s])
                          for j in slots]
                    if not any(w > 0 for w in ws):
                        continue
                    lhsT = vol_sb[:, s * D:(s + 1) * D]
                    if ELIDE_LDW:
                        nc.tensor.ldweights(lhsT)
                    for j, w in zip(slots, ws):
                        if w == 0:
                            continue
                        b = bank_of[j]
                        colp = block_of[j] * U + int(
                            offs[BLOCK_BASE_SLOT[SLOTS[j][1]]][s]
                            if SLOTS[j][2] is None else offs[j][s])
                        inst = nc.tensor.matmul(
                            ps[b][:, colp:colp + w],
                            lhsT=lhsT,
                            rhs=rhs_ap(j, s),
                            start=(i == first_of_bank[b]),
                            stop=(i == last_of_bank[b]),
                        )
                        if ELIDE_LDW:
                            inst.ldweights = False
                        i += 1
                # ---- drain this phase's banks (DVE + ACT), then flush on the
                # HWDGE rings (all stream dma_starts already in the rings).
                pb = (0, 1) if phase == 0 else (2, 3)
                fl_eng = [nc.sync, nc.scalar]
                nq = 0
                for b in pb:
                    n = len(banks[b]) * U
                    ob = out_base[banks[b][0]]
                    h = n // 2
                    for o0, o1 in ((0, h), (h, n)):
                        if nq % 2 == 0:
                            nc.vector.tensor_scalar_mul(
                                out_sb[:, ob + o0:ob + o1],
                                ps[b][:, o0:o1], float(DT))
                        else:
                            nc.scalar.mul(out_sb[:, ob + o0:ob + o1],
                                          ps[b][:, o0:o1], float(DT))
                        fl_eng[nq % 2].dma_start(
                            out_d[:, ob + o0:ob + o1],
                            out_sb[:, ob + o0:ob + o1])
                        nq += 1

    nc.compile()
    meta = dict(nc=nc, Ms=Ms, axes=axes, widths=widths, offs=offs,
                lay0=lay0, lay1=lay1, order=order, out_base=out_base)
    _COMPILED[key] = meta
    return meta


def kernel(vol, angles):
    import os
    from concourse.bass_utils import run_bass_kernel_spmd

    vol = np.asarray(vol, dtype=np.float32)
    angles = np.asarray(angles, dtype=np.float32)
    meta = _get_compiled(angles)
    nc = meta["nc"]

    volA = vol[0, 0].reshape(S, H * D).astype(ml_dtypes.bfloat16)
    volB = np.ascontiguousarray(vol[0, 0].transpose(1, 0, 2)).reshape(
        S, H * D).astype(ml_dtypes.bfloat16)
    in_maps = []
    for c in range(NCORES):
        m0, m1 = _pack_core(meta["Ms"], meta["axes"], meta["widths"],
                            meta["offs"], meta["lay0"], meta["lay1"], c)
        in_maps.append({"volA": volA, "volB": volB, "m0": m0, "m1": m1})

    res = run_bass_kernel_spmd(nc, in_maps, core_ids=list(range(NCORES)))
    global _LAST_RES
    _LAST_RES = res
    full = np.empty((1, 1, U, A, V), np.float32)
    for c, r in enumerate(res.results):
        rc = r["out"].astype(np.float32)    # [v, pos*128 + u]
        for p, j in enumerate(meta["order"]):
            k = SLOTS[j][0][c]
            full[0, 0, :, k, :] = rc[:, p * U:(p + 1) * U].T
    return full
